# revision 26
# baseline (speedup 1.0000x reference)
"""v9: bf16 + 3/4-fp8 DoubleRow scores + DVE-offloaded normalizer.

On top of v7 (below), the score phase contracts its first 768 of 1024
d-dims in fp8e4 DoubleRow: three K=256 matmuls at 0.5 cycles/row
replace six K=128 bf16 matmuls per accumulation group (-96 matmuls/
core, PE busy 164.6 -> 135.2 us in the scheduler sim, 134.7 us/rep
marginal). The softmax makes the scores the most quantization-tolerant
phase; quantizing 3/4 of the contraction raises end-to-end rel err
from 3.5e-3 to 1.649e-2 -- under the 2e-2 gate, and DETERMINISTIC: the
host pre-quantizes the fp8/bf16 operands, products are exact in fp32
PSUM, and the hosted numpy simulation of the full pipeline reproduces
the hardware result exactly at every precision configuration tried
(sim 1.6490e-2 = hw 1.649e-2; sim 1.3672e-2 = hw 1.367e-2 at half-fp8;
sim 3.540e-3 vs hw 3.544e-3 at bf16-only). fp8 on the full contraction
(1.92e-2) or anywhere else (A/C/D: >= 3e-2) was rejected.

The DoubleRow layout costs nothing extra on the host: the stationary
x operand packs as [ki, j, ko, t] whose column order (j, ko, t) equals
the existing pretiled XS layout, so the fp8 half ships as a dtype-cast
of the same array; the moving TT pairs are written by phase A's PSUM
eviction directly into [ki, ko, q] pair-tiles.

v7 notes: bf16 everywhere + DVE-offloaded softmax normalizer.

Same fused algorithm as v4 (M = (FQ)(FK)^T folded on host; device does
S^T = x @ (xq M)^T, es = exp(S^T/32) resident, G^T = x^T @ es / Z,
out = G @ V), but:

  * all matmul operands are bf16: same 1 cycle/row PE rate as float32r,
    but half the HBM traffic (~14 MB/core), half the SBUF footprint,
    and LDWEIGHTS gets the automatic 2x fast-weight-load path (FWL is
    fp32-disabled), which matters because phase C swaps its stationary
    operand on every accumulation step.
  * all DMA sources are pre-tiled on the host into the exact SBUF
    layout, so every transfer is a contiguous [128, free] row-slice at
    2 KB/partition -- no strided rearrange descriptors.
  * phase C hoists the qc loop inside the tt loop: one stationary load
    feeds both 512-wide moving chunks (128 LDW instead of 256).
  * Z = sum_t es: the 16-tile reduction runs as an fp32 elementwise
    chain on the otherwise-idle VectorE; only the final 128-partition
    fold uses the PE (2 ones-matmuls instead of 32: -6.4 us PE).
  * all tile pools are open across phases AND reps, so each phase's
    stream buffers prefetch during the previous phase and PSUM bank
    rotation never hands out a bank that is still being evicted
    (the scheduler sim showed 2-3 us PE stalls at every phase boundary
    with nested pools).
  * output stores go on the scalar DMA queue so input loads on the
    sync queue never serialize behind them.

Precision: bf16 inputs with fp32 PSUM give ~3.5e-3 end-to-end rel err
(bf16 x bf16 products are exact in fp32), comfortably under the 2e-2
gate; fp8/DoubleRow was measured at >=1.9e-2 end-to-end and rejected.
770 matmuls/core x 512 cols = 164.6 us of PE streaming at 2.4 GHz; the
scheduler sim shows a 165 us/rep marginal cost with zero PE gaps
(v4 baseline: 186 us sim, plus unmodeled fp32 LDWEIGHTS exposure).
Interleaved A/B on hardware: v7 = 194 us vs v4 = 231 us (same-process
medians; absolute numbers drift ~20% with chip power state).

Core c = (batch b=c//2, query-half h=c%2).
"""

import os
import sys

import numpy as np
import ml_dtypes

sys.path.insert(0, "/opt/trn_rl_repo")

import concourse.bass as bass  # noqa: E402
import concourse.tile as tile  # noqa: E402
from concourse import bacc, mybir  # noqa: E402
from concourse.bass_utils import run_bass_kernel_spmd  # noqa: E402

D = 1024
S = 2048
B = 4
H = 1024
P = 128
DT = D // P       # 8
TT = S // P       # 16
QT = H // P       # 8
NCH = 512
SCALE = 1.0 / 32.0

f32 = mybir.dt.float32
bf16 = mybir.dt.bfloat16
fp8 = mybir.dt.float8e4
DR = mybir.MatmulPerfMode.DoubleRow
EXP = mybir.ActivationFunctionType.Exp
BF16 = ml_dtypes.bfloat16
FP8 = ml_dtypes.float8_e4m3
NDR = 3            # DoubleRow groups: d-blocks 0..5 (d < 768) in fp8
DIN0 = 2 * NDR     # first bf16 d-block in phase B
NBF = D - DIN0 * P # bf16 tail of the score contraction (256)

_cache = {}
last_run_info = {}


def _build(repeat=1):
    nc = bacc.Bacc("TRN2", target_bir_lowering=False, debug=False, num_devices=8)

    # host-pretiled inputs; every DMA below is a contiguous row-slice
    xq_d = nc.dram_tensor("XQ", [D, H], bf16, kind="ExternalInput").ap()
    wm_d = nc.dram_tensor("WM", [D, D], bf16, kind="ExternalInput").ap()
    xs8_d = nc.dram_tensor("XS8", [S, NDR, 2, P], fp8, kind="ExternalInput").ap()
    xsb_d = nc.dram_tensor("XSB", [S, NBF], bf16, kind="ExternalInput").ap()
    xn_d = nc.dram_tensor("XN", [D, S], bf16, kind="ExternalInput").ap()
    v_d = nc.dram_tensor("V", [D, D], bf16, kind="ExternalInput").ap()
    ones_d = nc.dram_tensor("onesP", [P, P], bf16, kind="ExternalInput").ap()
    out = nc.dram_tensor("out", [H, D], f32, kind="ExternalOutput").ap()

    outs = [out] + [
        nc.dram_tensor(f"out_rep{r}", [H, D], f32).ap() for r in range(1, repeat)
    ]

    with tile.TileContext(nc) as tc:
      for _rep in range(repeat):
        out = outs[_rep]
        # every pool is open across all phases of the rep: consecutive
        # phases' stream buffers get disjoint SBUF ranges, so phase N+1's
        # DMAs prefetch during phase N instead of hitting a WAR hazard on
        # a reused range (nested per-phase pools cost 2-3 us PE stalls at
        # each phase boundary in the scheduler sim). Pools are scoped to
        # the rep, not the whole build: a cross-rep pool graph made the
        # tile scheduler ~8x slower on the repeated timing builds for a
        # ~1.3 us/rep win.
        with (
            tc.tile_pool(name="es", bufs=TT) as es_pool,
            tc.tile_pool(name="gxt", bufs=DT) as gxt_pool,
            tc.tile_pool(name="osb", bufs=2) as o_pool,
            tc.tile_pool(name="misc", bufs=1) as misc_pool,
            tc.tile_pool(name="ps", bufs=8, space="PSUM") as ps_pool,
            tc.tile_pool(name="ttx", bufs=DT) as tt_pool,
            tc.tile_pool(name="xq", bufs=DT) as xq_pool,
            tc.tile_pool(name="w", bufs=3) as w_pool,
            tc.tile_pool(name="xs", bufs=3) as xs_pool,
            tc.tile_pool(name="xnt", bufs=3) as xnt_pool,
            tc.tile_pool(name="vw", bufs=DT) as vw_pool,
        ):
            ones = misc_pool.tile([P, P], bf16, name="ones")
            nc.sync.dma_start(ones[:], ones_d[:])
            zbc = misc_pool.tile([P, H], f32, name="zbc")
            ztmp = [
                misc_pool.tile([P, H], f32, name=f"ztmp{i}") for i in range(2)
            ]
            zsum = misc_pool.tile([P, H], bf16, name="zsum")

            es = [
                es_pool.tile([P, H], bf16, tag="es", name=f"es{i}")
                for i in range(TT)
            ]
            gxt = [
                gxt_pool.tile([P, H], bf16, tag="gxt", name=f"gxt{i}")
                for i in range(DT)
            ]

            # TT d-blocks 0..3 live as fp8 DoubleRow pair-tiles [ki, ko, q];
            # blocks 4..7 stay bf16.
            tt8 = [
                tt_pool.tile([P, 2, H], fp8, tag="tt8", name=f"tt8{j}")
                for j in range(NDR)
            ]
            ttx = [
                tt_pool.tile([P, H], bf16, tag="ttx", name=f"ttx{i}")
                for i in range(DIN0, DT)
            ]

            # ---- phase A: TT = (xq @ M)^T -----------------------------
            def load_xq(dt_i):
                t = xq_pool.tile([P, H], bf16, tag="xq", name=f"xq{dt_i}")
                nc.sync.dma_start(t[:], xq_d[dt_i * P:(dt_i + 1) * P, :])
                return t

            def load_wm(dout):
                wt = w_pool.tile([P, D], bf16, tag="w", name=f"wm{dout}")
                nc.sync.dma_start(wt[:], wm_d[dout * P:(dout + 1) * P, :])
                return wt

            xq = [load_xq(0)]
            wt0 = load_wm(0)
            xq.extend(load_xq(i) for i in range(1, DT))

            for dout in range(DT):
                wt = wt0 if dout == 0 else load_wm(dout)
                accs = [
                    ps_pool.tile([P, NCH], f32, tag="acc", name=f"acc{i}")
                    for i in range(H // NCH)
                ]
                for din in range(DT):
                    for qc in range(H // NCH):
                        nc.tensor.matmul(
                            accs[qc][:],
                            wt[:, din * P:(din + 1) * P],
                            xq[din][:, qc * NCH:(qc + 1) * NCH],
                            start=(din == 0),
                            stop=(din == DT - 1),
                        )
                for qc in range(H // NCH):
                    if dout < DIN0:
                        dst = tt8[dout // 2][:, dout % 2, qc * NCH:(qc + 1) * NCH]
                    else:
                        dst = ttx[dout - DIN0][:, qc * NCH:(qc + 1) * NCH]
                    nc.vector.tensor_copy(dst, accs[qc][:])

            # ---- phase B: es = exp(S^T/32), resident ------------------
            # Z = sum_t es[t, q]: the 16-tile part of the reduction runs
            # as an fp32 elementwise chain on the (mostly idle) VectorE;
            # only the final 128-partition fold needs the PE (2 matmuls
            # against ones instead of 32).
            for tt_i in range(TT):
                # stationary x tiles: d<512 as fp8 DoubleRow pairs
                # [ki, j, ko, t] (the (j, ko, t) column order is exactly
                # the pretiled XS layout), d>=512 as bf16
                xs8 = xs_pool.tile([P, NDR, 2, P], fp8, tag="xs8", name="xs8")
                nc.sync.dma_start(
                    xs8[:], xs8_d[tt_i * P:(tt_i + 1) * P, :, :, :]
                )
                xsb = xs_pool.tile([P, NBF], bf16, tag="xsb", name="xsb")
                nc.sync.dma_start(xsb[:], xsb_d[tt_i * P:(tt_i + 1) * P, :])
                acc_s = [
                    ps_pool.tile([P, NCH], f32, tag="acc", name=f"accs{i}")
                    for i in range(H // NCH)
                ]
                for j in range(NDR):
                    for qc in range(H // NCH):
                        nc.tensor.matmul(
                            acc_s[qc][:],
                            xs8[:, j, :, :],
                            tt8[j][:, :, qc * NCH:(qc + 1) * NCH],
                            start=(j == 0),
                            stop=False,
                            perf_mode=DR,
                        )
                for din in range(DIN0, DT):
                    for qc in range(H // NCH):
                        nc.tensor.matmul(
                            acc_s[qc][:],
                            xsb[:, (din - DIN0) * P:(din - DIN0 + 1) * P],
                            ttx[din - DIN0][:, qc * NCH:(qc + 1) * NCH],
                            start=False,
                            stop=(din == DT - 1),
                        )
                for qc in range(H // NCH):
                    nc.scalar.activation(
                        es[tt_i][:, qc * NCH:(qc + 1) * NCH],
                        acc_s[qc][:],
                        EXP,
                        scale=SCALE,
                    )
                if tt_i == 0:
                    nc.vector.tensor_copy(ztmp[0][:], es[0][:])
                else:
                    nc.vector.tensor_add(
                        ztmp[tt_i % 2][:], ztmp[(tt_i - 1) % 2][:], es[tt_i][:]
                    )
            nc.vector.tensor_copy(zsum[:], ztmp[(TT - 1) % 2][:])

            # ---- phase C: G^T = x^T @ es, normalized by 1/Z -----------
            # The 2 Z-fold matmuls are emitted after C's first 32-matmul
            # accumulation group: by then the DVE zsum chain has long
            # finished, so the PE never waits on it (emitting them right
            # at the B->C boundary cost a ~3us PE stall in the sim).
            for dt_o in range(DT):
                xnt = xnt_pool.tile([P, S], bf16, tag="xnt", name="xnt")
                nc.sync.dma_start(xnt[:], xn_d[dt_o * P:(dt_o + 1) * P, :])
                pg = [
                    ps_pool.tile([P, NCH], f32, tag="acc", name=f"pg{i}")
                    for i in range(H // NCH)
                ]
                for tt_i in range(TT):
                    for qc in range(H // NCH):
                        nc.tensor.matmul(
                            pg[qc][:],
                            xnt[:, tt_i * P:(tt_i + 1) * P],
                            es[tt_i][:, qc * NCH:(qc + 1) * NCH],
                            start=(tt_i == 0),
                            stop=(tt_i == TT - 1),
                        )
                if dt_o == 0:
                    acc_z = [
                        ps_pool.tile([P, NCH], f32, tag="acc", name=f"accz{i}")
                        for i in range(H // NCH)
                    ]
                    for qc in range(H // NCH):
                        nc.tensor.matmul(
                            acc_z[qc][:],
                            ones[:],
                            zsum[:, qc * NCH:(qc + 1) * NCH],
                        )
                        nc.vector.reciprocal(
                            zbc[:, qc * NCH:(qc + 1) * NCH], acc_z[qc][:]
                        )
                for qc in range(H // NCH):
                    nc.vector.tensor_mul(
                        gxt[dt_o][:, qc * NCH:(qc + 1) * NCH],
                        pg[qc][:],
                        zbc[:, qc * NCH:(qc + 1) * NCH],
                    )

            # ---- phase D: out = G @ V ---------------------------------
            vw = []
            for din in range(DT):
                t = vw_pool.tile([P, D], bf16, tag="vw", name=f"vw{din}")
                nc.sync.dma_start(t[:], v_d[din * P:(din + 1) * P, :])
                vw.append(t)

            for qt in range(QT):
                acc_o = [
                    ps_pool.tile([P, NCH], f32, tag="acc", name=f"acco{i}")
                    for i in range(D // NCH)
                ]
                for dt_o in range(DT):
                    lhs = gxt[dt_o][:, qt * P:(qt + 1) * P]
                    for dc in range(D // NCH):
                        nc.tensor.matmul(
                            acc_o[dc][:],
                            lhs,
                            vw[dt_o][:, dc * NCH:(dc + 1) * NCH],
                            start=(dt_o == 0),
                            stop=(dt_o == DT - 1),
                        )
                o_sb = o_pool.tile([P, D], f32, tag="osb", name="osb")
                for dc in range(D // NCH):
                    nc.vector.tensor_copy(
                        o_sb[:, dc * NCH:(dc + 1) * NCH], acc_o[dc][:]
                    )
                # store on the scalar queue so the next rep's input loads
                # on the sync queue don't serialize behind these stores
                nc.scalar.dma_start(out[qt * P:(qt + 1) * P, :], o_sb[:])

    nc.compile()
    return nc


def _host_prep(x, F, Q, K, V):
    eye = np.eye(D, dtype=np.float32)
    if np.array_equal(F, eye):
        FQ, FK = Q, K
    else:
        FQ, FK = F @ Q, F @ K
    M = (FQ.astype(np.float64) @ FK.astype(np.float64).T).astype(np.float32)
    # WM[do*128+p, dt*128+m] = M[dt*128+p, do*128+m]
    WM = np.ascontiguousarray(
        M.astype(BF16).reshape(8, 128, 8, 128).transpose(2, 1, 0, 3)
        .reshape(D, D)
    )
    Vb = np.ascontiguousarray(V.astype(BF16))
    onesP = np.ones((P, P), dtype=BF16)
    maps = []
    for c in range(8):
        b, h = divmod(c, 2)
        xb = x[b].astype(BF16)                      # [S, D]
        xr = xb.reshape(16, 128, 8, 128)            # [tt, t, dt, d]
        # XS[tt*128+p, dt*128+t'] = x[tt*128+t', dt*128+p]; the d<512
        # half ships as fp8 (DoubleRow stationary: cols (j, ko, t) ==
        # (dt, t)), the d>=512 half as bf16
        XS = xr.transpose(0, 3, 2, 1).reshape(S, D)
        XS8 = np.ascontiguousarray(
            x[b].reshape(16, 128, 8, 128)[:, :, :2 * NDR, :]
            .transpose(0, 3, 2, 1).reshape(S, NDR, 2, 128).astype(FP8)
        )
        XSB = np.ascontiguousarray(XS[:, 2 * NDR * 128:])
        # XN[do*128+p, tt*128+m] = x[tt*128+p, do*128+m]
        XN = np.ascontiguousarray(xr.transpose(2, 1, 0, 3).reshape(D, S))
        # XQ = x^T for this core's query half: [D, H]
        XQ = np.ascontiguousarray(xb[h * H:(h + 1) * H, :].T)
        maps.append(
            {"XQ": XQ, "WM": WM, "XS8": XS8, "XSB": XSB, "XN": XN,
             "V": Vb, "onesP": onesP}
        )
    return maps


def kernel(x, F, Q, K, V):
    x = np.ascontiguousarray(np.asarray(x, dtype=np.float32))
    F = np.ascontiguousarray(np.asarray(F, dtype=np.float32))
    Q = np.ascontiguousarray(np.asarray(Q, dtype=np.float32))
    K = np.ascontiguousarray(np.asarray(K, dtype=np.float32))
    V = np.ascontiguousarray(np.asarray(V, dtype=np.float32))

    if "nc" not in _cache:
        _cache["nc"] = _build()
    nc = _cache["nc"]

    res = run_bass_kernel_spmd(nc, _host_prep(x, F, Q, K, V), list(range(8)))
    last_run_info["exec_time_ns"] = res.exec_time_ns

    out = np.empty((B, S, D), dtype=np.float32)
    for c in range(8):
        b, h = divmod(c, 2)
        out[b, h * H:(h + 1) * H, :] = res.results[c]["out"]
    return out
